# revision 2
# baseline (speedup 1.0000x reference)
"""Trainium2 Bass kernel for nn_NodeNetwork (GNN message passing + MLP + L2 norm).

Data-parallel over nodes: 500000 nodes -> 62500/core across 8 NeuronCores.
Per core, nodes are processed in supertiles of P*C nodes (node = r0 + p*C + c)
so every DMA moves large per-partition-contiguous chunks:
  - message: 4MB DMAs, 32KB contiguous per partition
  - features/global_features/out: 1MB DMAs, 8KB contiguous per partition
The DEG-16 mailbox sum runs on the TensorEngine as accumulating transposes
(PSUM accumulation), the MLP runs feature-major in bf16, and the final L2
norm runs node-major after transposing back on the TensorEngine.
"""

import numpy as np

F = 128
DEG = 16
H1 = 256
H2 = 256
OUT = 128
N_CORES = 8

_NC_CACHE = {}


def build(n_nodes, n_cores=N_CORES, cfg="safe"):
    import concourse.bacc as bacc
    import concourse.mybir as mybir
    import concourse.tile as tile
    import concourse.masks as masks
    from contextlib import ExitStack

    f32 = mybir.dt.float32
    f32r = mybir.dt.float32r
    bf16 = mybir.dt.bfloat16
    ACTF = mybir.ActivationFunctionType

    # In "f32r" mode the PE transposes run with float32r operands
    # (1.5 vs 2.0 cycles/row) via AP bitcast; tiles stay float32.
    if cfg == "f32r":
        rc = lambda ap: ap.bitcast(f32r)
    else:
        rc = lambda ap: ap

    # ---- supertile decomposition: (row0, P partitions, C nodes/partition) ----
    sts = []
    r = 0
    while n_nodes - r >= 2048:
        sts.append((r, 128, 16))
        r += 2048
    while n_nodes - r >= 512:
        sts.append((r, 128, 4))
        r += 512
    while n_nodes - r >= 128:
        sts.append((r, 128, 1))
        r += 128
    if n_nodes - r:
        sts.append((r, n_nodes - r, 1))

    nc = bacc.Bacc(
        "TRN2", target_bir_lowering=False, debug=False, num_devices=n_cores
    )
    msg_d = nc.dram_tensor("message", [n_nodes, DEG * F], f32, kind="ExternalInput").ap()
    feat_d = nc.dram_tensor("features", [n_nodes, F], f32, kind="ExternalInput").ap()
    glob_d = nc.dram_tensor(
        "global_features", [n_nodes, F], f32, kind="ExternalInput"
    ).ap()
    w1_d = nc.dram_tensor("W1", [3 * F, H1], f32, kind="ExternalInput").ap()
    b1_d = nc.dram_tensor("b1", [H1], f32, kind="ExternalInput").ap()
    w2_d = nc.dram_tensor("W2", [H1, H2], f32, kind="ExternalInput").ap()
    b2_d = nc.dram_tensor("b2", [H2], f32, kind="ExternalInput").ap()
    w3_d = nc.dram_tensor("W3", [H2, OUT], f32, kind="ExternalInput").ap()
    b3_d = nc.dram_tensor("b3", [OUT], f32, kind="ExternalInput").ap()
    out_d = nc.dram_tensor("out", [n_nodes, OUT], f32, kind="ExternalOutput").ap()

    with tile.TileContext(nc) as tc, ExitStack() as ctx:
        cpool = ctx.enter_context(tc.tile_pool(name="consts", bufs=1))
        mpool = ctx.enter_context(tc.tile_pool(name="msg", bufs=3))
        ipool = ctx.enter_context(tc.tile_pool(name="inputs", bufs=2))
        xtpool = ctx.enter_context(tc.tile_pool(name="xt", bufs=2))
        hpool = ctx.enter_context(tc.tile_pool(name="hid", bufs=2))
        npool = ctx.enter_context(tc.tile_pool(name="norm", bufs=2))
        opool = ctx.enter_context(tc.tile_pool(name="outp", bufs=2))
        ps_x = ctx.enter_context(tc.tile_pool(name="ps_x", bufs=2, space="PSUM"))
        ps_mm = ctx.enter_context(tc.tile_pool(name="ps_mm", bufs=2, space="PSUM"))
        ps_t = ctx.enter_context(tc.tile_pool(name="ps_t", bufs=1, space="PSUM"))

        # --- constants ---
        identf = cpool.tile([128, 128], f32, tag="identf")
        masks.make_identity(nc, identf[:])

        w1f = cpool.tile([128, 3 * H1], f32, tag="w1f")
        nc.sync.dma_start(w1f[:], w1_d.rearrange("(k p) m -> p k m", p=128))
        w2f = cpool.tile([128, 2 * H2], f32, tag="w2f")
        nc.sync.dma_start(w2f[:], w2_d.rearrange("(k p) m -> p k m", p=128))
        w3f = cpool.tile([128, 2 * OUT], f32, tag="w3f")
        nc.sync.dma_start(w3f[:], w3_d.rearrange("(k p) m -> p k m", p=128))
        w1sb = cpool.tile([128, 3 * H1], bf16, tag="w1")
        nc.vector.tensor_copy(w1sb[:], w1f[:])
        w2sb = cpool.tile([128, 2 * H2], bf16, tag="w2")
        nc.vector.tensor_copy(w2sb[:], w2f[:])
        w3sb = cpool.tile([128, 2 * OUT], bf16, tag="w3")
        nc.vector.tensor_copy(w3sb[:], w3f[:])
        b1sb = cpool.tile([128, 2], f32, tag="b1")
        nc.sync.dma_start(b1sb[:], b1_d.rearrange("(m p) -> p m", p=128))
        b2sb = cpool.tile([128, 2], f32, tag="b2")
        nc.sync.dma_start(b2sb[:], b2_d.rearrange("(m p) -> p m", p=128))
        b3sb = cpool.tile([128, 1], f32, tag="b3")
        nc.sync.dma_start(b3sb[:], b3_d.rearrange("(m p) -> p m", p=128))

        for (r0, P, C) in sts:
            # --- per-supertile loads of features/global_features ---
            featt = ipool.tile([128, C * F], f32, tag="featt")
            nc.sync.dma_start(
                featt[:P].rearrange("p (c f) -> p c f", c=C),
                feat_d[r0 : r0 + P * C].rearrange("(p c) f -> p c f", p=P),
            )
            globt = ipool.tile([128, C * F], f32, tag="globt")
            nc.sync.dma_start(
                globt[:P].rearrange("p (c f) -> p c f", c=C),
                glob_d[r0 : r0 + P * C].rearrange("(p c) f -> p c f", p=P),
            )
            outsb = opool.tile([128, C * F], f32, tag="outsb")

            # chunks of up to 4 blocks (cn nodes per partition each)
            chunks = [(c0, min(4, C - c0)) for c0 in range(0, C, 4)]
            for (c0, cn) in chunks:
                NV = cn * 128  # column stride per x-group
                NW = (cn - 1) * 128 + P  # valid columns (nodes) in this chunk
                # --- message load: cn*8KB contiguous per partition ---
                msgt = mpool.tile([128, cn * DEG * F], f32, tag="msgt")
                nc.sync.dma_start(
                    msgt[:P].rearrange("p (c j) -> p c j", c=cn),
                    msg_d[r0 : r0 + P * C].rearrange("(p c) j -> p c j", p=P)[
                        :, c0 : c0 + cn, :
                    ],
                )

                # --- x^T pieces via PE transposes (agg accumulates over DEG).
                # All matmuls into one PSUM bank form a single accumulation
                # group (start only on the first, stop only on the last);
                # fresh columns overwrite via unset has_written bits.
                xt = xtpool.tile([128, 3 * NV], bf16, tag="xt")
                pxa = ps_x.tile([128, NV], f32, tag="px")
                for b in range(cn):
                    for d in range(DEG):
                        nc.tensor.matmul(
                            rc(pxa[:, b * 128 : b * 128 + P]),
                            rc(msgt[:P, (b * DEG + d) * F : (b * DEG + d + 1) * F]),
                            rc(identf[:P, :P]),
                            is_transpose=True,
                            start=(b == 0 and d == 0),
                            stop=(b == cn - 1 and d == DEG - 1),
                        )
                nc.vector.tensor_copy(xt[:, 0:NW], pxa[:, 0:NW])

                pxf = ps_x.tile([128, NV], f32, tag="px")
                for b in range(cn):
                    nc.tensor.matmul(
                        rc(pxf[:, b * 128 : b * 128 + P]),
                        rc(featt[:P, (c0 + b) * F : (c0 + b + 1) * F]),
                        rc(identf[:P, :P]),
                        is_transpose=True,
                        start=(b == 0),
                        stop=(b == cn - 1),
                    )
                nc.vector.tensor_copy(xt[:, NV : NV + NW], pxf[:, 0:NW])

                pxg = ps_x.tile([128, NV], f32, tag="px")
                for b in range(cn):
                    nc.tensor.matmul(
                        rc(pxg[:, b * 128 : b * 128 + P]),
                        rc(globt[:P, (c0 + b) * F : (c0 + b + 1) * F]),
                        rc(identf[:P, :P]),
                        is_transpose=True,
                        start=(b == 0),
                        stop=(b == cn - 1),
                    )
                nc.vector.tensor_copy(xt[:, 2 * NV : 2 * NV + NW], pxg[:, 0:NW])

                # --- layer 1: [384 -> 256], relu (feature-major, bf16) ---
                h1 = hpool.tile([128, 2 * NV], bf16, tag="h1")
                for m in range(2):
                    pmm = ps_mm.tile([128, NV], f32, tag="pm1")
                    for k in range(3):
                        nc.tensor.matmul(
                            pmm[:, 0:NW],
                            w1sb[:, k * H1 + m * 128 : k * H1 + (m + 1) * 128],
                            xt[:, k * NV : k * NV + NW],
                            start=(k == 0),
                            stop=(k == 2),
                        )
                    nc.scalar.activation(
                        h1[:, m * NV : m * NV + NW],
                        pmm[:, 0:NW],
                        ACTF.Relu,
                        bias=b1sb[:, m : m + 1],
                    )

                # --- layer 2: [256 -> 256], relu ---
                h2 = hpool.tile([128, 2 * NV], bf16, tag="h2")
                for m in range(2):
                    pmm = ps_mm.tile([128, NV], f32, tag="pm2")
                    for k in range(2):
                        nc.tensor.matmul(
                            pmm[:, 0:NW],
                            w2sb[:, k * H2 + m * 128 : k * H2 + (m + 1) * 128],
                            h1[:, k * NV : k * NV + NW],
                            start=(k == 0),
                            stop=(k == 1),
                        )
                    nc.scalar.activation(
                        h2[:, m * NV : m * NV + NW],
                        pmm[:, 0:NW],
                        ACTF.Relu,
                        bias=b2sb[:, m : m + 1],
                    )

                # --- layer 3: [256 -> 128] + b3 ---
                pm3 = ps_t.tile([128, NV], f32, tag="pm3")
                for k in range(2):
                    nc.tensor.matmul(
                        pm3[:, 0:NW],
                        w3sb[:, k * OUT : (k + 1) * OUT],
                        h2[:, k * NV : k * NV + NW],
                        start=(k == 0),
                        stop=(k == 1),
                    )
                o3 = hpool.tile([128, NV], f32, tag="o3")
                nc.scalar.activation(
                    o3[:, 0:NW], pm3[:, 0:NW], ACTF.Identity, bias=b3sb[:, 0:1]
                )

                # --- transpose back to node-major ---
                pout = ps_t.tile([128, cn * F], f32, tag="pout")
                for b in range(cn):
                    nc.tensor.matmul(
                        rc(pout[:P, b * F : (b + 1) * F]),
                        rc(o3[:, b * 128 : b * 128 + P]),
                        rc(identf[:, :]),
                        is_transpose=True,
                        start=(b == 0),
                        stop=(b == cn - 1),
                    )

                # --- row L2 norm ---
                sqt = npool.tile([128, F], f32, tag="sqt", bufs=1)
                nsq = npool.tile([128, cn], f32, tag="nsq")
                for b in range(cn):
                    nc.scalar.activation(
                        sqt[:P],
                        pout[:P, b * F : (b + 1) * F],
                        ACTF.Square,
                        accum_out=nsq[:P, b : b + 1],
                    )
                nv = npool.tile([128, cn], f32, tag="nv")
                nc.scalar.activation(nv[:P], nsq[:P], ACTF.Sqrt)
                nve = npool.tile([128, cn], f32, tag="nve")
                nc.vector.tensor_scalar_add(nve[:P], nv[:P], 1e-8)
                ri = npool.tile([128, cn], f32, tag="ri")
                nc.vector.reciprocal(ri[:P], nve[:P])
                for b in range(cn):
                    nc.vector.tensor_scalar_mul(
                        outsb[:P, (c0 + b) * F : (c0 + b + 1) * F],
                        pout[:P, b * F : (b + 1) * F],
                        ri[:P, b : b + 1],
                    )

            # --- store ---
            nc.scalar.dma_start(
                out_d[r0 : r0 + P * C].rearrange("(p c) f -> p c f", p=P),
                outsb[:P].rearrange("p (c f) -> p c f", c=C),
            )

    nc.compile()
    return nc


CFG = "safe"


def _get_nc(n_nodes, n_cores):
    import os

    cfg = os.environ.get("BASS_NODE_CFG", CFG)
    key = (n_nodes, n_cores, cfg)
    if key not in _NC_CACHE:
        _NC_CACHE[key] = build(n_nodes, n_cores, cfg=cfg)
    return _NC_CACHE[key]


def kernel(message, features, global_features, W1, b1, W2, b2, W3, b3):
    from concourse.bass_utils import run_bass_kernel_spmd

    n = message.shape[0]
    assert n % N_CORES == 0
    npc = n // N_CORES

    nc = _get_nc(npc, N_CORES)

    def shard(a, shape):
        return np.ascontiguousarray(
            np.asarray(a, dtype=np.float32).reshape((N_CORES,) + shape)
        )

    msg = shard(message, (npc, DEG * F))
    feat = shard(features, (npc, F))
    glob = shard(global_features, (npc, F))
    w1 = np.ascontiguousarray(np.asarray(W1, np.float32))
    w2 = np.ascontiguousarray(np.asarray(W2, np.float32))
    w3 = np.ascontiguousarray(np.asarray(W3, np.float32))
    bb1 = np.ascontiguousarray(np.asarray(b1, np.float32))
    bb2 = np.ascontiguousarray(np.asarray(b2, np.float32))
    bb3 = np.ascontiguousarray(np.asarray(b3, np.float32))

    in_maps = [
        {
            "message": msg[i],
            "features": feat[i],
            "global_features": glob[i],
            "W1": w1,
            "b1": bb1,
            "W2": w2,
            "b2": bb2,
            "W3": w3,
            "b3": bb3,
        }
        for i in range(N_CORES)
    ]
    res = run_bass_kernel_spmd(nc, in_maps, list(range(N_CORES))).results
    return np.concatenate([res[i]["out"] for i in range(N_CORES)], axis=0)
